# revision 18
# baseline (speedup 1.0000x reference)
"""AnomalyAttention on 8 Trainium2 NeuronCores (Bass/Tile), data-parallel over batch.

Problem: B,L,H,E = 8,1024,8,64
  score  = (1/sqrt(E)) * einsum('blhe,bshe->bhls', Q, K)
  gauss  = kappa/sig_l * exp(-(l-s)^2 / (2 sig_l^2))       (kappa = 1/sqrt(2 pi))
  G_V    = softmax(score, s) @ G_values
  L_V    = softmax(score + gauss, s) @ L_values

Device strategy (per core = one batch element, loop over 8 heads):
  Transposed layout P[s, l]; L-path factored as encg*(Vl^T P) + band correction
  (see _host_prep).  The PE array executes two concurrent matmul streams when
  instructions sit on disjoint array tiles (row- or col-groups) with different
  PSUM regions; the emission schedule is built around that:
    - QK pairs (head-e rows 0:64 / head-o rows 64:128) in 4-deep bursts
      across the two score banks -> ~2x stream rate.
    - Z one-hot chains col-cycle 4 strips, head-o shifted 2 strips so
      adjacent Z's always use disjoint strips.
    - AV (M=128, full array) is inherently serial; the two heads' chains
      alternate banks to avoid same-bank RMW stalls.
    - Band matmuls (M=65) emitted between Z items (strip-3 Z's overlap them).
  The L*L exp splits ScalarE (true exp, 5/8 chunks) / VectorE (Schraudolph
  fast-exp, 3/8).  Band multiplies run on GpSimd (SBUF->SBUF).  PE warm-up
  matmuls at body start trip the HAM clock gate (1.2 -> 2.4 GHz) before the
  real work arrives.  Final Z DMA is split across 4 engine queues.
"""

import math
import numpy as np
import ml_dtypes

BF16 = ml_dtypes.bfloat16
B, L, H, E = 8, 1024, 8, 64
NCH = L // 128          # 8 s-chunks of 128
BAND = 16               # gauss band halfwidth (W < 3e-7 beyond; bf16-invisible)
WW = 128 + 2 * BAND     # 160: W tile width in l per s-chunk
N_CORES = 8
DVE_KS = (1, 3, 5)      # s-chunks whose exp runs on VectorE (fast-exp)

LOG2E = 1.4426950408889634
FX_SIGMA = 0.0579       # Schraudolph minimax centering
FX_A = 128.0 * LOG2E / math.sqrt(E)       # folds the 1/sqrt(E) score scale
FX_B = (127.0 - FX_SIGMA) * 128.0

_NC_CACHE = {}


def _build_nc():
    if "nc" in _NC_CACHE:
        return _NC_CACHE["nc"]
    import concourse.bacc as bacc
    import concourse.tile as tile
    from concourse import mybir
    from concourse.tile import add_dep_helper

    f32 = mybir.dt.float32
    bf16 = mybir.dt.bfloat16
    i16 = mybir.dt.int16

    nc = bacc.Bacc()
    qkt_d = nc.declare_dram_parameter("qkt", [4, 128, 2 * L], bf16, isOutput=False)
    # vgl[h, :, k, 0:64] = V_g chunk, 64:128 = V_l chunk, col 128 = ones
    vgl_d = nc.declare_dram_parameter("vgl", [H, 128, NCH, 129], bf16, isOutput=False)
    # wband2[i, :, k, p, :]: gauss band W for head 2i+p
    wbd_d = nc.declare_dram_parameter("wband", [4, 128, NCH, 2, WW], bf16,
                                      isOutput=False)
    # out[h, half, 0] = [128, 512]: rows 0:64 Vg^T P, rows 64:128 Vl^T P
    # out[h, half, 1] = [65, 512]: band correction (+ band Z row 64)
    outGL_d = nc.declare_dram_parameter("outGL", [H, 2, 128, 512], bf16, isOutput=True)
    outB_d = nc.declare_dram_parameter("outB", [H, 2, 65, 512], bf16, isOutput=True)
    # one-hot Z accumulator: row 32j+m = partial Z for (h,half)=m over chunks
    # distributed to col-strips j; host sums the 4 j-streams.
    outZ_d = nc.declare_dram_parameter("outZ", [128, 512], f32, isOutput=True)

    with tile.TileContext(nc) as tc:
        with (
            tc.tile_pool(name="ones_p", bufs=1) as ones_p,
            tc.tile_pool(name="qkt_p", bufs=2) as qkt_p,
            tc.tile_pool(name="v_p", bufs=4) as v_p,
            tc.tile_pool(name="w_p", bufs=2) as w_p,
            tc.tile_pool(name="pg_p", bufs=20) as pg_p,
            tc.tile_pool(name="mb_p", bufs=34) as mb_p,
            tc.tile_pool(name="stg_p", bufs=3) as stg_p,
            tc.tile_pool(name="sc_p", bufs=2, space="PSUM") as sc_p,
            tc.tile_pool(name="acc_p", bufs=1, space="PSUM") as acc_p,
        ):
            # sliding one-hot weight bank for Z: col 15 is ones; variant m
            # (= 4i+2p+half) is zw[:, 15-m:31-m]
            zw = ones_p.tile([128, 31], bf16, tag="zw", bufs=1)
            nc.vector.memset(zw, 0.0)
            nc.vector.memset(zw[:, 15:16], 1.0)
            zrow = ones_p.tile([1, 512], bf16, tag="zrow", bufs=1)
            nc.vector.memset(zrow, 0.0)
            # one persistent Z bank for the whole kernel; warm-up matmuls
            # write it first, the real clear (start=True) comes after them,
            # then all Z matmuls accumulate with start=False.
            accZ = acc_p.tile([128, 512], f32, tag="accZ", bufs=1)

            def warm_mm(n=256, off=0):
                nc.tensor.matmul(out=accZ[:, off:off + n],
                                 lhsT=zrow[:, 0:128],
                                 rhs=zrow[:, off:off + n], start=True,
                                 stop=True, skip_group_check=True)

            state = {}

            def emit_qk_burst(i, q, kk):
                st = state[i]
                qt = st["qt"]
                for k in (kk, kk + 1):
                    sc = sc_p.tile([128, 2, 512], f32, tag="sc", bufs=2,
                                   name="sc")
                    st["sck"][k] = sc
                    for p in range(2):
                        pslc = slice(64 * p, 64 * p + 64)
                        nc.tensor.matmul(
                            out=sc[:, p, :],
                            lhsT=qt[pslc, 512 + 128 * k:512 + 128 * (k + 1)],
                            rhs=(qt[pslc, 0:512] if q == 0 else
                                 qt[pslc, 1536:2048]),
                            start=True, stop=True,
                            tile_position=(64 * p, 0),
                        )

            def emit_exp(i, q, k):
                st = state[i]
                sc = st["sck"][k]
                wb = st["wb"]
                pgk = pg_p.tile([128, 2, 512], bf16, tag="pg", bufs=40,
                                name="pgk")
                if k in DVE_KS:
                    nc.vector.tensor_scalar(
                        out=pgk.bitcast(i16), in0=sc,
                        scalar1=FX_A, scalar2=FX_B,
                        op0=mybir.AluOpType.mult,
                        op1=mybir.AluOpType.add,
                    )
                else:
                    nc.scalar.activation(
                        out=pgk, in_=sc,
                        func=mybir.ActivationFunctionType.Exp,
                        scale=1.0 / math.sqrt(E),
                    )
                st["pgq"][k][q] = pgk
                # band product piece for this l-half (both heads at once)
                a0 = max(0, 128 * k - BAND)
                b0 = min(L, 128 * k + 128 + BAND)
                a = max(a0, 512 * q)
                bb = min(b0, 512 * (q + 1))
                if bb > a:
                    if st["mb"][k] is None:
                        st["mb"][k] = mb_p.tile([128, 2, WW], bf16,
                                                tag="mb", bufs=24,
                                                name="mbk")
                    woff = a - (128 * k - BAND)
                    nc.gpsimd.tensor_mul(
                        out=st["mb"][k][:, :, woff:woff + (bb - a)],
                        in0=pgk[:, :, a - 512 * q:bb - 512 * q],
                        in1=wb[:, k, :, woff:woff + (bb - a)],
                    )

            def sweep_prologue(i, q):
                if q == 0 and i in state:
                    return  # prologue already prefetched
                if q == 0:
                    qt = qkt_p.tile([128, 2 * L], bf16, tag="qkt", bufs=2,
                                    name="qt")
                    # layout [Q-lo | K | Q-hi]: the first transfer alone
                    # (Q-lo + key chunks 0,1) unblocks burst 0; rest follow.
                    nc.gpsimd.dma_start(out=qt[:, 0:768],
                                          in_=qkt_d.ap()[i][:, 0:768])
                    nc.sync.dma_start(out=qt[:, 768:1536],
                                      in_=qkt_d.ap()[i][:, 768:1536])
                    nc.sync.dma_start(out=qt[:, 1536:2048],
                                      in_=qkt_d.ap()[i][:, 1536:2048])
                    wb = w_p.tile([128, NCH, 2, WW], bf16, tag="wb", bufs=2,
                                  name="wb")
                    nc.scalar.dma_start(out=wb, in_=wbd_d.ap()[i])
                    state[i] = {
                        "qt": qt, "wb": wb, "mb": [None] * NCH,
                        "sck": [None] * NCH,
                        "pgq": [[None, None] for _ in range(NCH)],
                        "vgls": [],
                    }
                # q==1 queries prefetched in the q==0 prologue

            def sweep_epilogue(i, q):
                if q == 0:
                    st = state[i]
                    for p in range(2):
                        h = 2 * i + p
                        vgl = v_p.tile([128, NCH, 129], bf16, tag=f"vgl{p}",
                                       bufs=2, name="vgl")
                        nc.scalar.dma_start(out=vgl, in_=vgl_d.ap()[h])
                        st["vgls"].append(vgl)

            def av_half_items(i, half):
                """Closures for the AV/Z/band/evac work of one (pair, half)."""
                st = state[i]
                pgq, mb = st["pgq"], st["mb"]
                items = []
                accGL = [None, None]
                accB = [None]

                def mk_av(p, k):
                    def f():
                        nc.tensor.matmul(out=accGL[p],
                                         lhsT=st["vgls"][p][:, k, 0:128],
                                         rhs=pgq[k][half][:, p, :],
                                         start=(k == 0), stop=(k == NCH - 1))
                    return f

                def alloc_acc():
                    accGL[0] = acc_p.tile([128, 512], f32, tag="accGL",
                                          bufs=2, name="accGLe")
                    accGL[1] = acc_p.tile([128, 512], f32, tag="accGL",
                                          bufs=2, name="accGLo")
                items.append(alloc_acc)
                for k in range(NCH):
                    items.append(mk_av(0, k))
                    items.append(mk_av(1, k))

                def mk_evac_gl(p):
                    def f():
                        h = 2 * i + p
                        stgGL = stg_p.tile([128, 512], bf16, tag="stgGL",
                                           bufs=3, name="stgGL")
                        if p == 0:
                            nc.vector.tensor_copy(out=stgGL, in_=accGL[p])
                        else:
                            nc.scalar.copy(out=stgGL, in_=accGL[p])
                        nc.sync.dma_start(out=outGL_d.ap()[h, half],
                                          in_=stgGL)
                    return f


                h0 = half * 512
                spans = []
                for k in range(NCH):
                    a = max(0, 128 * k - BAND, h0)
                    bb = min(L, 128 * k + 128 + BAND, h0 + 512)
                    if bb > a:
                        spans.append((k, a, bb))

                def mk_band(p):
                    def f():
                        accB[0] = acc_p.tile([65, 512], f32, tag="accB",
                                             bufs=1, name="accB")
                        b_first = None
                        for jj, (k, a, bb) in enumerate(spans):
                            off = a - (128 * k - BAND)
                            mmb = nc.tensor.matmul(
                                out=accB[0][:, a - h0:bb - h0],
                                lhsT=st["vgls"][p][:, k, 64:129],
                                rhs=mb[k][:, p, off:off + (bb - a)],
                                start=(jj == 0), stop=(jj == len(spans) - 1),
                                skip_group_check=True,
                            )
                            if jj == 0:
                                b_first = mmb
                            else:
                                add_dep_helper(mmb.ins, b_first.ins,
                                               reason="bank clear first")
                    return f

                def mk_evac_b(p, eng):
                    def f():
                        h = 2 * i + p
                        stgB = stg_p.tile([65, 512], bf16, tag="stgB",
                                          bufs=3, name="stgB")
                        if eng == "scalar":
                            nc.scalar.copy(out=stgB, in_=accB[0])
                        else:
                            nc.vector.tensor_copy(out=stgB, in_=accB[0])
                        nc.scalar.dma_start(out=outB_d.ap()[h, half],
                                            in_=stgB)
                    return f

                def mk_z(p, k, j):
                    def f():
                        m = 4 * i + 2 * p + half
                        nc.tensor.matmul(out=accZ[32 * j:32 * j + 16, :],
                                         lhsT=zw[:, 15 - m:31 - m],
                                         rhs=pgq[k][half][:, p, :],
                                         start=False, stop=False,
                                         tile_position=(0, 32 * j),
                                         skip_group_check=True)
                    return f

                items.append(mk_z(0, 3, 3))
                items.append(mk_band(0))
                items.append(mk_z(0, 7, 3))
                zseq = [(0, 0), (1, 0), (0, 1), (1, 2), (0, 2), (1, 3),
                        (0, 4), (1, 4), (0, 5), (1, 6), (0, 6), (1, 7)]
                for idx, (p, k) in enumerate(zseq):
                    items.append(mk_z(p, k, idx % 3))
                items.append(mk_evac_b(0, "vector"))
                items.append(mk_z(1, 1, 3))
                items.append(mk_band(1))
                items.append(mk_z(1, 5, 3))
                items.append(mk_evac_gl(0))
                items.append(mk_evac_gl(1))
                items.append(mk_evac_b(1, "scalar"))
                return items

            def emit_sweep(i, q, av_items):
                sweep_prologue(i, q)
                # split av_items into 4 groups placed after each burst's exps
                ngrp = (len(av_items) + 3) // 4 if av_items else 0
                for bi, kk in enumerate((0, 2, 4, 6)):
                    emit_qk_burst(i, q, kk)
                    emit_exp(i, q, kk)
                    emit_exp(i, q, kk + 1)
                    for it in av_items[bi * ngrp:(bi + 1) * ngrp]:
                        it()
                sweep_epilogue(i, q)

            # software pipeline: av items of half n interleave into the QK
            # sweep of half n+1; the first sweep gets PE warm-up matmuls
            # instead (trips the HAM clock gate during the DMA wait).
            sweep_prologue(0, 0)   # qt/wband DMAs first: min time-to-PE
            warm_mm()
            warm_mm()
            nc.tensor.matmul(out=accZ, lhsT=zrow[:, 0:128], rhs=zrow,
                             start=True, stop=False, skip_group_check=True)
            pend = []
            first_filler = [
                (lambda o: (lambda: warm_mm(256, o)))(256 * (j % 2))
                for j in range(8)
            ]
            for i in range(4):
                for q in range(2):
                    if q == 1 and i < 3:
                        sweep_prologue(i + 1, 0)  # prefetch next pair's qt
                    if not pend:
                        emit_sweep(i, q, first_filler)
                    else:
                        emit_sweep(i, q, pend.pop(0))
                    pend.append(av_half_items(i, q))
            for items in pend:
                for it in items:
                    it()
            # Z evacuation: copy then 4-way split DMA across engine queues
            stgZ = stg_p.tile([128, 512], f32, tag="stgZ", bufs=1, name="stgZ")
            nc.vector.tensor_copy(out=stgZ, in_=accZ)
            nc.sync.dma_start(out=outZ_d.ap()[:, 0:172], in_=stgZ[:, 0:172])
            nc.scalar.dma_start(out=outZ_d.ap()[:, 172:344],
                                in_=stgZ[:, 172:344])
            nc.gpsimd.dma_start(out=outZ_d.ap()[:, 344:512],
                                in_=stgZ[:, 344:512])
    nc.compile()
    _NC_CACHE["nc"] = nc
    return nc


def _host_prep(G_queries, G_keys, G_values, L_values, sigma):
    """Build per-core input dicts + host-side encg [L, H] per core."""
    inv_sqrt_2pi = 1.0 / math.sqrt(2.0 * math.pi)
    sig = sigma.astype(np.float32)
    sig = 1.0 / (1.0 + np.exp(-5.0 * sig.astype(np.float64)))
    sig = (sig + 1e-05).astype(np.float32)
    sig = (np.float32(3.0) ** sig) - np.float32(1.0)          # [B, L, H]
    c = inv_sqrt_2pi / sig.astype(np.float64)                  # [B, L, H]
    encg = np.exp(-c)                                          # [B, L, H]
    nhi = 1.0 / (2.0 * sig.astype(np.float64) ** 2)

    in_maps = []
    aux = []
    for b in range(B):
        qkt = np.empty((4, 128, 2 * L), BF16)
        for h in range(H):
            i, p = divmod(h, 2)
            qkt[i, 64 * p:64 * p + 64, 0:512] = G_queries[b, :512, h, :].T
            qkt[i, 64 * p:64 * p + 64, 512:1536] = G_keys[b, :, h, :].T
            qkt[i, 64 * p:64 * p + 64, 1536:2048] = G_queries[b, 512:, h, :].T
        # packed V: [V_g | V_l | ones] per chunk, layout [H, 128, NCH, 129]
        vgl = np.empty((H, 128, NCH, 129), BF16)
        gv = G_values[b].reshape(NCH, 128, H, E)   # [k, p, h, e]
        lv = L_values[b].reshape(NCH, 128, H, E)
        vgl[:, :, :, 0:64] = np.ascontiguousarray(gv.transpose(2, 1, 0, 3))
        vgl[:, :, :, 64:128] = np.ascontiguousarray(lv.transpose(2, 1, 0, 3))
        vgl[..., 128] = 1.0
        # W band tiles [4, 128, NCH, 2, WW] (pairs of heads)
        wband = np.zeros((4, 128, NCH, 2, WW), BF16)
        s_off = np.arange(128)
        j_off = np.arange(WW)
        for k in range(NCH):
            s_idx = 128 * k + s_off                  # [128]
            l_idx = 128 * k - BAND + j_off           # [WW]
            valid = (l_idx >= 0) & (l_idx < L)
            lvx = np.clip(l_idx, 0, L - 1)
            d = l_idx[None, :] - s_idx[:, None]      # [128, WW]
            band_ok = (np.abs(d) <= BAND) & valid[None, :]
            for h in range(H):
                i, p = divmod(h, 2)
                ch = c[b, lvx, h][None, :]
                g = ch * np.exp(-(d.astype(np.float64) ** 2) * nhi[b, lvx, h][None, :])
                W = np.exp(g - ch) - encg[b, lvx, h][None, :]
                W[~band_ok] = 0.0
                wband[i, :, k, p, :] = W.astype(np.float32)
        in_maps.append({"qkt": np.asarray(qkt),
                        "vgl": np.asarray(vgl),
                        "wband": np.asarray(wband)})
        aux.append(encg[b])  # [L, H]
    return in_maps, aux


def _host_post(outs, aux):
    G_V = np.empty((B, L, H, E), np.float32)
    L_V = np.empty((B, L, H, E), np.float32)
    for b in range(B):
        oGL = outs[b]["outGL"].astype(np.float64)  # [H, 2, 128, 512]
        oB = outs[b]["outB"].astype(np.float64)    # [H, 2, 65, 512]
        oZ = outs[b]["outZ"].astype(np.float64)    # [128, 512] one-hot packed
        # Z for (h, half) = m = 4i+2p+half: sum of the 4 col-strip streams
        zm = oZ[0:16] + oZ[32:48] + oZ[64:80] + oZ[96:112]   # [16, 512]
        for h in range(H):
            i, p = divmod(h, 2)
            GLt = np.concatenate([oGL[h, 0], oGL[h, 1]], axis=1)  # [128, L]
            Bt = np.concatenate([oB[h, 0], oB[h, 1]], axis=1)     # [65, L]
            Z = np.concatenate([zm[4 * i + 2 * p + 0],
                                zm[4 * i + 2 * p + 1]])           # [L]
            e = aux[b][:, h]  # [L]
            G_V[b, :, h, :] = (GLt[0:64] / Z).T
            Lnum = GLt[64:128] * e[None, :] + Bt[:64]
            Lden = Z * e + Bt[64]
            L_V[b, :, h, :] = (Lnum / Lden).T
    return G_V, L_V


def kernel(G_queries, G_keys, G_values, L_values, sigma):
    from concourse.bass_utils import run_bass_kernel_spmd

    args = [np.asarray(x, dtype=np.float32) for x in
            (G_queries, G_keys, G_values, L_values, sigma)]
    nc = _build_nc()
    in_maps, aux = _host_prep(*args)
    res = run_bass_kernel_spmd(nc, in_maps, core_ids=list(range(N_CORES)),
                               trace=False)
    return _host_post(res.results, aux)
